# revision 35
# baseline (speedup 1.0000x reference)
"""Trainium2 Bass kernel for a transformer encoder layer.

Computes, for x = enc_inputs [B=16, S=512, D=1024]:
    q,k,v = x@Wq+bq, x@Wk+bk, x@Wv+bv          (H=16 heads, dk=dv=64)
    attn  = softmax(q@k.T/sqrt(dk) + mask)      -> output [B,H,S,S]
    ctx   = LN((attn@v)@Wo + bo + x)
    out   = LN(relu(ctx@w1+b1)@w2 + b2 + ctx)   -> output [B,S,D]

Sharding: data-parallel over batch, 2 batch elements per core x 8 cores.
All GEMMs run as float32r (TF32-class: 8-bit exp / 11-bit mantissa) on the
PE array at full rate; weights are pre-rounded to fp32r on the host so the
device matmuls are exact over the rounded operands.
"""
import numpy as np

import concourse.bass as bass
from concourse import bacc
import concourse.mybir as mybir
import concourse.tile as tile
from concourse.bass_utils import run_bass_kernel_spmd

F32 = mybir.dt.float32
F32R = mybir.dt.float32r
AF = mybir.ActivationFunctionType
ALU = mybir.AluOpType

P = 128
S = 512
D = 1024
H = 16
DK = 64
DFF = 4096
NB = 2            # batch elements per core
NCORES = 8
SQT = S // P      # 4 seq tiles
DCT = D // P      # 8 d-model chunks
FFB = DFF // 512  # 8 ff blocks
EPS = 1e-6
MASK_NEG = -30000.0   # exp() underflows to exactly 0.0 in fp32


def _round_fp32r(x: np.ndarray) -> np.ndarray:
    """Round fp32 -> fp32r (round-to-nearest-even at 11 mantissa bits)."""
    u = np.ascontiguousarray(x).view(np.uint32).astype(np.uint64)
    u = (u + 0x7FF + ((u >> 12) & 1)) & 0xFFFFF000
    return u.astype(np.uint32).view(np.float32).reshape(x.shape)


def build():
    nc = bacc.Bacc(trn_type="TRN2")

    # ---- DRAM I/O ------------------------------------------------------
    x_d = nc.dram_tensor("x", [NB, S, D], F32, kind="ExternalInput")
    xpb_d = nc.dram_tensor("xpb", [NB, S, D], F32, kind="ExternalInput")  # x + bo
    mb_d = nc.dram_tensor("maskbias", [NB, S], F32, kind="ExternalInput")
    wq_d = nc.dram_tensor("Wq", [D, D], F32R, kind="ExternalInput")
    wk_d = nc.dram_tensor("Wk", [D, D], F32R, kind="ExternalInput")
    wv_d = nc.dram_tensor("Wv", [D, D], F32R, kind="ExternalInput")
    wo_d = nc.dram_tensor("Wo", [D, D], F32R, kind="ExternalInput")
    w1_d = nc.dram_tensor("w1", [D, DFF], F32R, kind="ExternalInput")
    w2_d = nc.dram_tensor("w2", [DFF, D], F32R, kind="ExternalInput")
    bq_d = nc.dram_tensor("bq", [D], F32, kind="ExternalInput")   # pre-scaled by 1/8
    bk_d = nc.dram_tensor("bk", [D], F32, kind="ExternalInput")
    bv_d = nc.dram_tensor("bv", [D], F32, kind="ExternalInput")
    b1_d = nc.dram_tensor("b1", [DFF], F32, kind="ExternalInput")
    b2_d = nc.dram_tensor("b2", [D], F32R, kind="ExternalInput")
    ones_d = nc.dram_tensor("ones_d", [1, P], F32R, kind="ExternalInput")
    ident_d = nc.dram_tensor("ident_d", [P, P], F32R, kind="ExternalInput")
    out_d = nc.dram_tensor("out", [NB, S, D], F32, kind="ExternalOutput")
    attn_d = nc.dram_tensor("attn", [NB, H, S, S], F32, kind="ExternalOutput")

    wq_c = wq_d.ap().rearrange("(c p) n -> p c n", p=P)   # [128, 8, 1024]
    wk_c = wk_d.ap().rearrange("(c p) n -> p c n", p=P)
    wv_c = wv_d.ap().rearrange("(c p) n -> p c n", p=P)
    wo_c = wo_d.ap().rearrange("(c p) n -> p c n", p=P)
    w1_c = w1_d.ap().rearrange("(c p) n -> p c n", p=P)   # [128, 8, 4096]
    w2_c = w2_d.ap().rearrange("(c p) n -> p c n", p=P)   # [128, 32, 1024]

    with tile.TileContext(nc) as tc:
        # ---- pools (strict LIFO alloc/release order) -------------------
        const = tc.alloc_tile_pool(name="const", bufs=1)
        ps_gemm = tc.alloc_tile_pool(name="ps_gemm", bufs=2, space="PSUM")
        ps_score = tc.alloc_tile_pool(name="ps_score", bufs=2, space="PSUM")
        ps_scoreT = tc.alloc_tile_pool(name="ps_scoreT", bufs=2, space="PSUM")
        ps_ctx = tc.alloc_tile_pool(name="ps_ctx", bufs=2, space="PSUM")
        ps_tr = ps_scoreT   # x/y transposes run outside the attention phase
        p_out2 = tc.alloc_tile_pool(name="p_out2", bufs=1)
        p_ctxT = tc.alloc_tile_pool(name="p_ctxT", bufs=1)
        p_ab = tc.alloc_tile_pool(name="p_ab", bufs=1)

        # ---- constants -------------------------------------------------
        ones_sb = const.tile([1, P], F32R, name="ones_sb")
        nc.sync.dma_start(ones_sb, ones_d.ap())
        ident = const.tile([P, P], F32R, name="ident")
        nc.sync.dma_start(ident, ident_d.ap())
        mb_ap = mb_d.ap()
        mb_bc = const.tile([P, NB, S], F32, name="mb_bc")
        nc.gpsimd.dma_start(
            out=mb_bc,
            in_=bass.AP(tensor=mb_ap.tensor, offset=0, ap=[[0, P]] + list(mb_ap.ap)),
        )
        # mask bias with s_k on partitions: mbT[p, b, c] = maskbias[b, c*128+p]
        mbT = const.tile([P, NB, SQT], F32, name="mbT")
        for b in range(NB):
            nc.sync.dma_start(mbT[:, b, :],
                              mb_d.ap()[b].rearrange("(c p) -> p c", p=P))
        bv_ap = bv_d.ap()
        bv_bc = const.tile([P, D], F32, name="bv_bc")
        nc.gpsimd.dma_start(
            out=bv_bc,
            in_=bass.AP(tensor=bv_ap.tensor, offset=0, ap=[[0, P]] + list(bv_ap.ap)),
        )
        bq_sb = const.tile([P, DCT], F32, name="bq_sb")
        nc.sync.dma_start(bq_sb, bq_d.ap().rearrange("(t p) -> p t", p=P))
        bk_sb = const.tile([P, DCT], F32, name="bk_sb")
        nc.sync.dma_start(bk_sb, bk_d.ap().rearrange("(t p) -> p t", p=P))
        b1_sb = const.tile([P, DFF // P], F32, name="b1_sb")
        nc.sync.dma_start(b1_sb, b1_d.ap().rearrange("(t p) -> p t", p=P))
        b2_row = const.tile([1, D], F32R, name="b2_row")
        nc.sync.dma_start(b2_row, b2_d.ap().rearrange("(a d) -> a d", a=1))
        eps_sb = const.tile([P, 1], F32, name="eps_sb")
        nc.vector.memset(eps_sb, EPS)
        ones_col_sb = const.tile([P, H], F32R, name="ones_col_sb")
        nc.gpsimd.dma_start(
            out=ones_col_sb,
            in_=bass.AP(tensor=ones_d.ap().tensor, offset=0,
                        ap=[[0, P], [1, H]]))

        # ================= phase A: x -> xT, v ==========================
        p_av = tc.alloc_tile_pool(name="p_av", bufs=3)

        xT = {}   # xT[b][j] : [128(d), 512(s)] fp32r
        for b in range(NB):
            x_b = x_d.ap()[b].rearrange("(i p) d -> p i d", p=P)
            for j in range(DCT):
                xs = p_av.tile([P, SQT, P], F32, tag="xstrip",
                               name=f"xstrip_{b}_{j}")
                nc.sync.dma_start(xs, x_b[:, :, j * P:(j + 1) * P])
                pst = ps_tr.tile([P, SQT, P], F32, tag="sT", name=f"xtr_{b}_{j}")
                for i in range(SQT):
                    nc.tensor.transpose(
                        pst[:, i, :], xs[:, i, :], ident.bitcast(F32))
                dst = p_ab.tile([P, S], F32R, tag=f"xT_{b}_{j}", name=f"xT_{b}_{j}")
                nc.vector.tensor_copy(dst, pst.rearrange("p a q -> p (a q)"))
                xT.setdefault(b, {})[j] = dst

        # v[b][i] : [128(s_k), 16(head), 65] fp32r -- per-head V with a ones
        # column appended, so the ctx matmul also emits the softmax denom.
        v = {b: {} for b in range(NB)}
        for b in range(NB):
            for i in range(SQT):
                v[b][i] = p_ab.tile([P, H, DK + 1], F32R, tag=f"v_{b}_{i}",
                                    name=f"v_{b}_{i}")
                nc.vector.tensor_copy(
                    v[b][i][:, :, DK], ones_col_sb.bitcast(F32))
        for dh in range(2):
            wvt = p_av.tile([P, DCT, 512], F32R, tag="wvt", bufs=1,
                            name=f"wvt_{dh}")
            nc.sync.dma_start(wvt, wv_c[:, :, dh * 512:(dh + 1) * 512])
            for b in range(NB):
                for i in range(SQT):
                    ps = ps_gemm.tile([P, 512], F32, tag="g", name=f"vps_{dh}_{b}_{i}")
                    for c in range(DCT):
                        nc.tensor.matmul(
                            ps, xT[b][c][:, i * P:(i + 1) * P], wvt[:, c, :],
                            start=(c == 0), stop=(c == DCT - 1))
                    nc.vector.tensor_tensor(
                        v[b][i][:, dh * 8:(dh + 1) * 8, 0:DK],
                        ps.rearrange("p (h e) -> p h e", e=DK),
                        bv_bc[:, dh * 512:(dh + 1) * 512]
                        .rearrange("p (h e) -> p h e", e=DK),
                        ALU.add)
        p_av.release()

        # ============ phase B: q/k proj + attention (per head pair) =====
        p_abt = tc.alloc_tile_pool(name="p_abt", bufs=2)
        ctxT = {b: {} for b in range(NB)}

        for t in range(DCT):                      # head pair t -> heads 2t, 2t+1
            wqt = p_abt.tile([P, DCT, P], F32R, tag="wqt", name=f"wqt_{t}")
            nc.sync.dma_start(wqt, wq_c[:, :, t * P:(t + 1) * P])
            wkt = p_abt.tile([P, DCT, P], F32R, tag="wkt", name=f"wkt_{t}")
            nc.sync.dma_start(wkt, wk_c[:, :, t * P:(t + 1) * P])
            for b in range(NB):
                qps = ps_gemm.tile([P, S], F32, tag="g", name=f"qps_{t}_{b}")
                for c in range(DCT):
                    nc.tensor.matmul(qps, wqt[:, c, :], xT[b][c],
                                     start=(c == 0), stop=(c == DCT - 1))
                qTt = p_abt.tile([P, S], F32R, tag="qTt", name=f"qT_{t}_{b}")
                nc.scalar.activation(qTt, qps, AF.Identity,
                                     bias=bq_sb[:, t:t + 1], scale=0.125)
                kps = ps_gemm.tile([P, S], F32, tag="g", name=f"kps_{t}_{b}")
                for c in range(DCT):
                    nc.tensor.matmul(kps, wkt[:, c, :], xT[b][c],
                                     start=(c == 0), stop=(c == DCT - 1))
                kTt = p_abt.tile([P, S], F32R, tag="kTt", name=f"kT_{t}_{b}")
                nc.scalar.activation(kTt, kps, AF.Identity, bias=bk_sb[:, t:t + 1])

                ct = p_ctxT.tile([P, S], F32R, tag=f"ctxT_{t}_{b}",
                                 name=f"ctxT_{t}_{b}")
                collog = p_abt.tile([P, 2, SQT], F32, tag="clg", bufs=2,
                                    name=f"clg_{t}_{b}")
                for hh in range(2):               # head = 2t + hh
                    h = 2 * t + hh
                    row = hh * 64
                    # transposed scores -> exp -> unnormalized ctx^T + denom
                    cps = ps_ctx.tile([DK + 1, S], F32, tag="c",
                                      name=f"cps_{t}_{b}_{hh}")
                    for c in range(SQT):
                        stp = ps_scoreT.tile([P, S], F32, tag="sT",
                                             name=f"stp_{t}_{b}_{hh}_{c}")
                        nc.tensor.matmul(
                            stp, kTt[row:row + 64, c * P:(c + 1) * P],
                            qTt[row:row + 64, :],
                            start=True, stop=True, tile_position=(row, 0))
                        ptx = p_abt.tile([P, S], F32R, tag="ptx", bufs=4,
                                         name=f"ptx_{t}_{b}_{hh}_{c}")
                        nc.scalar.activation(ptx, stp, AF.Exp,
                                             bias=mbT[:, b, c:c + 1])
                        nc.tensor.matmul(
                            cps, v[b][c][:, h, :],
                            ptx, start=(c == 0), stop=(c == SQT - 1))
                    # rowlog = ln(denominator row); transpose to a column
                    rlg = p_abt.tile([1, S], F32, tag="rlg", bufs=3,
                                     name=f"rlg_{t}_{b}_{hh}")
                    nc.scalar.activation(rlg, cps[DK:DK + 1, :], AF.Ln)
                    lzp = ps_ctx.tile([P, SQT], F32, tag="c",
                                      name=f"lzp_{t}_{b}_{hh}")
                    for i in range(SQT):
                        nc.tensor.transpose(
                            lzp[:, i:i + 1], rlg[:, i * P:(i + 1) * P]
                            .bitcast(F32), ident.bitcast(F32)[0:1, 0:1])
                    nc.vector.tensor_scalar_mul(collog[:, hh, :], lzp, -1.0)
                    # normalize: ct_h = cps[0:64] * (1 / denom) rowwise
                    rcp = p_abt.tile([1, S], F32, tag="rcp", bufs=4,
                                     name=f"rcp_{t}_{b}_{hh}")
                    nc.vector.reciprocal(rcp, cps[DK:DK + 1, :])
                    rcb = p_abt.tile([DK, S], F32, tag="rcb", bufs=3,
                                     name=f"rcb_{t}_{b}_{hh}")
                    nc.gpsimd.partition_broadcast(rcb, rcp)
                    if hh == 0:
                        nc.vector.tensor_tensor(ct[0:64, :], cps[0:DK, :], rcb,
                                                ALU.mult)
                    else:
                        stg = p_abt.tile([DK, S], F32R, tag="stg",
                                         name=f"stg_{t}_{b}")
                        nc.vector.tensor_tensor(stg, cps[0:DK, :], rcb, ALU.mult)
                        nc.sync.dma_start(ct[64:128, :], stg)
                for hh in range(2):
                    # natural-layout scores -> P = exp(s + mask - logZ)
                    h = 2 * t + hh
                    row = hh * 64
                    for i in range(SQT):
                        sps = ps_score.tile([P, S], F32, tag="s",
                                            name=f"sps_{t}_{b}_{hh}_{i}")
                        nc.tensor.matmul(
                            sps, qTt[row:row + 64, i * P:(i + 1) * P],
                            kTt[row:row + 64, :],
                            start=True, stop=True, tile_position=(row, 0))
                        nc.vector.tensor_tensor(sps, sps, mb_bc[:, b, :], ALU.add)
                        pf = p_abt.tile([P, S], F32, tag="pfin", bufs=3,
                                        name=f"pfin_{t}_{b}_{hh}_{i}")
                        nc.scalar.activation(pf, sps, AF.Exp,
                                             bias=collog[:, hh, i:i + 1])
                        nc.sync.dma_start(
                            attn_d.ap()[b, h, i * P:(i + 1) * P, :], pf)
                ctxT[b][t] = ct
        p_abt.release()
        p_ab.release()

        # ============ phase C: Wo + residual + LN1, out2 = y ============
        p_c = tc.alloc_tile_pool(name="p_c", bufs=1)
        p_ct = tc.alloc_tile_pool(name="p_ct", bufs=3)

        wo_sb = {}
        for dh in range(2):
            wo_sb[dh] = p_c.tile([P, DCT, 512], F32R, tag=f"wo_{dh}",
                                 name=f"wo_{dh}")
            nc.sync.dma_start(wo_sb[dh], wo_c[:, :, dh * 512:(dh + 1) * 512])

        out2 = {b: {} for b in range(NB)}
        for b in range(NB):
            for i in range(SQT):
                lnin = p_ct.tile([P, D], F32, tag="lnin", name=f"lnin_{b}_{i}")
                xr = p_ct.tile([P, D], F32, tag="xr", name=f"xr_{b}_{i}")
                nc.sync.dma_start(xr, xpb_d.ap()[b, i * P:(i + 1) * P, :])
                for dh in range(2):
                    ps = ps_gemm.tile([P, 512], F32, tag="g",
                                      name=f"ops_{b}_{i}_{dh}")
                    for j in range(DCT):
                        nc.tensor.matmul(
                            ps, ctxT[b][j][:, i * P:(i + 1) * P], wo_sb[dh][:, j, :],
                            start=(j == 0), stop=(j == DCT - 1))
                    nc.vector.tensor_tensor(
                        lnin[:, dh * 512:(dh + 1) * 512], ps,
                        xr[:, dh * 512:(dh + 1) * 512], ALU.add)
                # LayerNorm (g=1, b=0)
                stats = p_ct.tile([P, 2, 6], F32, tag="stats", name=f"st1_{b}_{i}")
                for g in range(2):
                    nc.vector.bn_stats(stats[:, g, :], lnin[:, g * 512:(g + 1) * 512])
                mv = p_ct.tile([P, 2], F32, tag="mv", name=f"mv1_{b}_{i}")
                nc.vector.bn_aggr(mv, stats)
                std = p_ct.tile([P, 1], F32, tag="std", name=f"std1_{b}_{i}")
                nc.scalar.activation(std, mv[:, 1:2], AF.Sqrt, bias=eps_sb)
                nc.vector.reciprocal(std, std)
                o2 = p_out2.tile([P, D], F32, tag=f"out2_{b}_{i}",
                                 name=f"out2_{b}_{i}")
                nc.vector.tensor_scalar(o2, lnin, mv[:, 0:1], std,
                                        op0=ALU.subtract, op1=ALU.mult)
                out2[b][i] = o2
        p_ct.release()
        p_c.release()
        p_ctxT.release()

        # ============ phase D: y -> yT, FFN ============================
        p_yT = tc.alloc_tile_pool(name="p_yT", bufs=1)
        p_dt = tc.alloc_tile_pool(name="p_dt", bufs=2)

        yT = {b: {} for b in range(NB)}
        for b in range(NB):
            for j in range(DCT):
                pst = ps_tr.tile([P, SQT, P], F32, tag="sT", name=f"ytr_{b}_{j}")
                for i in range(SQT):
                    nc.tensor.transpose(
                        pst[:, i, :], out2[b][i][:, j * P:(j + 1) * P],
                        ident.bitcast(F32))
                yt = p_yT.tile([P, S], F32R, tag=f"yT_{b}_{j}", name=f"yT_{b}_{j}")
                nc.vector.tensor_copy(yt, pst.rearrange("p a q -> p (a q)"))
                yT[b][j] = yt

        for f in range(FFB):
            w1b = p_dt.tile([P, DCT, 512], F32R, tag="w1b", name=f"w1b_{f}")
            nc.sync.dma_start(w1b, w1_c[:, :, f * 512:(f + 1) * 512])
            w2b = p_dt.tile([P, 4, D], F32R, tag="w2b", bufs=1, name=f"w2b_{f}")
            nc.sync.dma_start(w2b, w2_c[:, f * 4:(f + 1) * 4, :])
            for b in range(NB):
                h1b = p_dt.tile([P, 4, 512], F32R, tag=f"h1_{b}", name=f"h1_{f}_{b}")
                for fs in range(4):
                    ps = ps_gemm.tile([P, 512], F32, tag="g",
                                      name=f"f1ps_{f}_{b}_{fs}")
                    for c in range(DCT):
                        nc.tensor.matmul(
                            ps, w1b[:, c, fs * P:(fs + 1) * P], yT[b][c],
                            start=(c == 0), stop=(c == DCT - 1))
                    nc.scalar.activation(h1b[:, fs, :], ps, AF.Relu,
                                         bias=b1_sb[:, f * 4 + fs:f * 4 + fs + 1])
                for i in range(SQT):
                    for dh in range(2):
                        ps = ps_gemm.tile([P, 512], F32, tag="g",
                                          name=f"f2ps_{f}_{b}_{i}_{dh}")
                        if f == 0:
                            nc.tensor.matmul(
                                ps, ones_sb, b2_row[:, dh * 512:(dh + 1) * 512],
                                start=True, stop=False)
                        for fs in range(4):
                            nc.tensor.matmul(
                                ps, h1b[:, fs, i * P:(i + 1) * P],
                                w2b[:, fs, dh * 512:(dh + 1) * 512],
                                start=(fs == 0 and f != 0), stop=(fs == 3))
                        nc.vector.tensor_tensor(
                            out2[b][i][:, dh * 512:(dh + 1) * 512],
                            out2[b][i][:, dh * 512:(dh + 1) * 512], ps, ALU.add)
        p_dt.release()
        p_yT.release()

        # ============ phase E: LN2 + store =============================
        p_e = tc.alloc_tile_pool(name="p_e", bufs=3)
        for b in range(NB):
            for i in range(SQT):
                r2 = out2[b][i]
                stats = p_e.tile([P, 2, 6], F32, tag="stats2", name=f"st2_{b}_{i}")
                for g in range(2):
                    nc.vector.bn_stats(stats[:, g, :], r2[:, g * 512:(g + 1) * 512])
                mv = p_e.tile([P, 2], F32, tag="mv2", name=f"mv2_{b}_{i}")
                nc.vector.bn_aggr(mv, stats)
                std = p_e.tile([P, 1], F32, tag="std2", name=f"std2_{b}_{i}")
                nc.scalar.activation(std, mv[:, 1:2], AF.Sqrt, bias=eps_sb)
                nc.vector.reciprocal(std, std)
                outf = p_e.tile([P, D], F32, tag="outf", name=f"outf_{b}_{i}")
                nc.vector.tensor_scalar(outf, r2, mv[:, 0:1], std,
                                        op0=ALU.subtract, op1=ALU.mult)
                nc.sync.dma_start(out_d.ap()[b, i * P:(i + 1) * P, :], outf)
        p_e.release()
        p_out2.release()
        ps_ctx.release()
        ps_tr.release()
        ps_score.release()
        ps_gemm.release()
        const.release()
    nc.finalize()
    return nc


_CACHE = {}


def _get_nc():
    if "nc" not in _CACHE:
        _CACHE["nc"] = build()
    return _CACHE["nc"]


def kernel(**inputs):
    enc = np.asarray(inputs["enc_inputs"], np.float32)       # [16, 512, 1024]
    mask = np.asarray(inputs["mask"])                        # [16, 512] int32
    B = enc.shape[0]

    Wq = _round_fp32r(np.asarray(inputs["Wq"], np.float32))
    Wk = _round_fp32r(np.asarray(inputs["Wk"], np.float32))
    Wv = _round_fp32r(np.asarray(inputs["Wv"], np.float32))
    Wo = _round_fp32r(np.asarray(inputs["Wo"], np.float32))
    w1 = _round_fp32r(np.asarray(inputs["w1"], np.float32))
    w2 = _round_fp32r(np.asarray(inputs["w2"], np.float32))
    bq = np.asarray(inputs["bq"], np.float32) * 0.125        # fold score scale
    bk = np.asarray(inputs["bk"], np.float32)
    bv = np.asarray(inputs["bv"], np.float32)
    bo = np.asarray(inputs["bo"], np.float32)
    b1 = np.asarray(inputs["b1"], np.float32)
    b2 = _round_fp32r(np.asarray(inputs["b2"], np.float32))
    g1 = np.asarray(inputs["ln1_g"], np.float32)
    be1 = np.asarray(inputs["ln1_b"], np.float32)
    g2 = np.asarray(inputs["ln2_g"], np.float32)
    be2 = np.asarray(inputs["ln2_b"], np.float32)
    assert np.all(g1 == 1) and np.all(be1 == 0), "kernel assumes ln1 g=1,b=0"
    assert np.all(g2 == 1) and np.all(be2 == 0), "kernel assumes ln2 g=1,b=0"

    xpb = (enc + bo[None, None, :]).astype(np.float32)
    # 0.0 where mask==1 (keep), MASK_NEG where mask==0 (pad)
    maskbias = ((mask.astype(np.float32) - 1.0) * (-MASK_NEG)).astype(np.float32)

    nc = _get_nc()
    shared = {
        "Wq": Wq, "Wk": Wk, "Wv": Wv, "Wo": Wo, "w1": w1, "w2": w2,
        "bq": bq, "bk": bk, "bv": bv, "b1": b1, "b2": b2,
        "ones_d": np.ones((1, P), np.float32),
        "ident_d": np.eye(P, dtype=np.float32),
    }
    in_maps = []
    for c in range(NCORES):
        sl = slice(c * NB, (c + 1) * NB)
        in_maps.append({
            "x": np.ascontiguousarray(enc[sl]),
            "xpb": np.ascontiguousarray(xpb[sl]),
            "maskbias": np.ascontiguousarray(maskbias[sl]),
            **shared,
        })
    res = run_bass_kernel_spmd(nc, in_maps, core_ids=list(range(NCORES)))
    _CACHE["last_result"] = res
    out = np.concatenate([r["out"] for r in res.results], axis=0)
    attn = np.concatenate([r["attn"] for r in res.results], axis=0)
    return out, attn
